# revision 86
# baseline (speedup 1.0000x reference)
"""DimeNet++-style GNN message passing on 8 trn2 NeuronCores.

Sharding: data-parallel over source atoms (i). Each core owns 64 source rows
of the 512x512 edge tensor; a per-block ReduceScatter hands each core the
aggregate for its own 64 nodes, the update MLP runs shard-local, and one
small AllReduce of the pooled head input reassembles the molecule means.

Key reformulation (unchanged from the baseline): for fixed source atom i and
channel h, the edge message silu(t_i[h] + g_h(d)) is a smooth scalar function
of the distance d alone; it is interpolated on M=12 uniform knots with a
piecewise-linear (hat) basis, so the N x N x H silu tensor collapses to knot
values Y (tiny) and distance-only hat slabs shared by all blocks.

New in this version — the serial inter-block chain never materializes the
residual stream X on the critical path.  X^b = x0 + sum_{b'<b} (h1_{b'} @
u2_{b'} + ub2_{b'}) is substituted into every consumer, so t_b, h1pre_b and
the output head each accumulate [x0 term + h1-history terms] in PSUM with
all composite weights (u2@w1x, u2@u1a, u2@out_w1) and every ub2 bias shift
folded on the host; only the single h1_{b-1} term sits on the serial chain.
The distance prologue computes d^2 + BIG*I in ONE matmul (|p|^2 rows and a
one-hot diag block ride the contraction dim), sqrt folds the 1/delta knot
scale, and the input DMAs are split across the SP and ACT HWDGE queues with
the big late-needed pack last.  Y-knot silus are triple-packed ([128,384]
per ACT op).  Aggregate staging for the ReduceScatter is one packed
[128,512] copy + one rearranged DMA per block; in the collective-free tsim
build the full-width chunk matmuls and staging are deferred into the next
block's silu bubble so the in-order PE reaches wu@S without them queued in
front.

PSUM discipline (measured on hw): a bank holds at most ONE open multi-matmul
accumulation window at a time — an interleaved start_tensor_calc on the same
bank resets the open window's partial (closed data persists).  Hence the
aggregate iterates chunk-major within its bank, the short-lived accumulators
(t / h1pre / head) time-share one bank with strictly disjoint windows, and
sT (+ the tsim head xo) has its own bank.  8 banks total: acc ring (2),
bc pair (2), a_ps ring (2), misc (1), sT (1).  GPSIMD cannot touch PSUM, so
PSUM->SBUF moves ride DVE (or ACT via AF.Copy for the last staging).
"""

import os
import numpy as np
import ml_dtypes


def tf32(x):
    x = np.ascontiguousarray(x, np.float32)
    u = x.view(np.uint32)
    return (((u + 0x1000 + ((u >> 13) & 1)) & 0xFFFFE000).astype(np.uint32)
            ).view(np.float32)

LAST_EXEC_NS = None

N = 512
H = 128
M = 12           # interpolation knots
NB = 4
NMOL = 16
NCORES = 8
SH = N // NCORES  # 64 source rows per core
BIG2 = 900.0     # added to diag of d^2 -> d ~ 30, outside knot range
NGM = M // 4     # knot groups per slab tile
NT = NGM * 2     # slab tiles: (M/4 m-groups) x (2 i-halves)


def bf16(x):
    return np.asarray(x, np.float32).astype(ml_dtypes.bfloat16)


def build_nc(inputs):
    import concourse.bacc as bacc
    import concourse.mybir as mybir
    import concourse.tile as tile

    f32 = mybir.dt.float32
    f32r = mybir.dt.float32r
    bf16d = mybir.dt.bfloat16

    an = np.asarray(inputs['atomic_numbers']).astype(np.int64)
    pos = np.asarray(inputs['positions']).astype(np.float64)
    batch = np.asarray(inputs['batch']).astype(np.int64)
    emb = np.asarray(inputs['emb']).astype(np.float32)
    centers = np.asarray(inputs['centers']).astype(np.float64)
    widths = np.asarray(inputs['widths']).astype(np.float64)
    msg_w1 = np.asarray(inputs['msg_w1']).astype(np.float64)
    msg_b1 = np.asarray(inputs['msg_b1']).astype(np.float64)
    msg_w2 = np.asarray(inputs['msg_w2']).astype(np.float64)
    msg_b2 = np.asarray(inputs['msg_b2']).astype(np.float64)
    upd_w1 = np.asarray(inputs['upd_w1']).astype(np.float64)
    upd_b1 = np.asarray(inputs['upd_b1']).astype(np.float64)
    upd_w2 = np.asarray(inputs['upd_w2']).astype(np.float64)
    upd_b2 = np.asarray(inputs['upd_b2']).astype(np.float64)
    out_w1 = np.asarray(inputs['out_w1']).astype(np.float64)
    out_b1 = np.asarray(inputs['out_b1']).astype(np.float64)
    out_w2 = np.asarray(inputs['out_w2']).astype(np.float64)
    out_b2 = np.asarray(inputs['out_b2']).astype(np.float64)

    # ---- host-side prep (index/weight transforms only) ----
    dmax = float(np.sqrt(3.0) * 1.0001)
    knots = np.linspace(0.0, dmax, M)
    delta = float(knots[1] - knots[0])

    rbf_k = np.exp(-((knots[:, None] - centers) ** 2) / (2.0 * widths ** 2))

    # cumulative ub2 shifts: X^b = x0 + sum_{b'<b} (h1_b' @ u2_b' + ub2_b')
    cumub2 = [np.zeros(H)]
    for b in range(NB):
        cumub2.append(cumub2[-1] + upd_b2[b])

    # G'_b = rbf(knots) @ W1r + b1 + (cumub2_b @ W1x)  (exact, f64)
    gall = np.concatenate(
        [rbf_k @ msg_w1[b, H:, :] + msg_b1[b][None, :]
         + (cumub2[b] @ msg_w1[b, :H, :])[None, :] for b in range(NB)],
        axis=1)                                        # [12, 512]

    # wu_b = -(w2_b @ u1r_b): folds aggregate->update (slabs hold -hat)
    wu = [-(msg_w2[b] @ upd_w1[b, H:, :]) for b in range(NB)]
    # h1 bias: ub1 + deg*b2 fold + cumub2 fold (deg == N-1 for every node)
    h1bias = np.stack(
        [upd_b1[b] + float(N - 1) * (msg_b2[b] @ upd_w1[b, H:, :])
         + cumub2[b] @ upd_w1[b, :H, :] for b in range(NB)], axis=1)  # [128,4]

    # composite weights for the X-free recurrences
    w2w1 = {(a, b): upd_w2[a] @ msg_w1[b, :H, :]
            for b in range(1, NB) for a in range(b)}
    uu = {(a, b): upd_w2[a] @ upd_w1[b, :H, :]
          for b in range(1, NB) for a in range(b)}
    uo = [upd_w2[b] @ out_w1 for b in range(NB)]       # [128, 64] each
    ob1p = out_b1 + cumub2[NB] @ out_w1                # [64]

    x0 = emb[np.clip(an, 0, 99)]                      # [N,H] f32

    counts = np.zeros(NMOL, np.float64)
    np.add.at(counts, batch, 1.0)
    poolT = np.zeros((N, NMOL), np.float32)
    poolT[np.arange(N), batch] = (1.0 / np.maximum(counts, 1.0))[batch].astype(np.float32)

    # slab bias vectors: partition p = 32*a + i' covers knot m = 4*gm + a
    negk = np.zeros((128, 2 * NGM), np.float32)
    for gm in range(NGM):
        for a in range(4):
            negk[32*a:32*(a+1), gm] = -knots[4*gm + a] / delta
            negk[32*a:32*(a+1), NGM + gm] = knots[4*gm + a] / delta

    f = np.float32
    ob1_col = np.zeros((128, 1), np.float32); ob1_col[:64, 0] = ob1p
    o2_col = np.zeros((128, 1), np.float32)
    o2_col[:64, 0] = out_w2[:, 0]; o2_col[64, 0] = 1.0
    ce_blocks = [negk, f(msg_w1[0, :H, :]), f(upd_w1[0, :H, :]),
                 f(h1bias), ob1_col, o2_col,
                 f(msg_w1[1, :H, :]), f(upd_w1[1, :H, :])]
    cE = np.concatenate(ce_blocks, axis=1)             # [128, 3+6+4*128]
    ca_blocks = ([f(wu[0])]
                 + [f(msg_w1[b, :H, :]) for b in (2, 3)]
                 + [f(upd_w1[b, :H, :]) for b in (2, 3)]
                 + [f(wu[1]), f(wu[2]), f(wu[3])]
                 + [f(out_w1)]
                 + [np.eye(128, dtype=np.float32)])
    cA = np.concatenate(ca_blocks, axis=1)
    # composite fold weights in bf16: every h1-history matmul then runs
    # at 1 cycle/row (h1 tiles are bf16 too), and the big f32 ca DMA
    # shrinks by ~60%
    pairs = [(0, 1), (0, 2), (1, 2), (0, 3), (1, 3), (2, 3)]
    wB = np.concatenate(
        [bf16(w2w1[p]) for p in pairs]
        + [bf16(uu[p]) for p in pairs]
        + [bf16(u) for u in uo], axis=1)               # [128, 1792] bf16

    # bf16 broadcast selectors: bsel (d-row broadcast), esel (t/G broadcast)
    bsel2 = np.zeros((64 + M, 2 * 128), np.float32)
    for gi in range(2):
        for a in range(4):
            bsel2[32*gi + np.arange(32), 128*gi + 32*a + np.arange(32)] = 1.0
    Esel = np.zeros((64 + M, NT * 128), np.float32)
    for gi in range(2):
        for gm in range(NGM):
            tix = NGM*gi + gm
            for a in range(4):
                Esel[64 + 4*gm + a, 128*tix + 32*a: 128*tix + 32*(a+1)] = 1.0
                Esel[32*gi + np.arange(32), 128*tix + 32*a + np.arange(32)] = 1.0
    bfcnp = np.concatenate([bsel2, Esel], axis=1)      # [76, 256+768]

    nall = np.sum(pos * pos, axis=1).astype(np.float64).reshape(1, N)

    per_core = []
    for c in range(NCORES):
        sl = slice(SH*c, SH*(c+1))
        # one-matmul d^2: contraction rows = [p | norm/one | one/norm | BIG*I]
        rhs = np.zeros((69, N))
        rhs[0:3] = pos.T
        rhs[3] = nall[0]
        rhs[4] = 1.0
        rhs[5:69, SH*c:SH*(c+1)] = np.eye(SH)
        lhs = np.zeros((69, SH))
        lhs[0:3] = -2.0 * pos[sl].T
        lhs[3] = 1.0
        lhs[4] = nall[0, sl]
        lhs[5:69] = BIG2 * np.eye(SH)
        x0c = np.zeros((128, SH + NMOL), np.float32)
        x0c[:, 0:SH] = x0[sl].T
        x0c[0:SH, SH:SH+NMOL] = poolT[sl]
        per_core.append({
            'c3': tf32(np.concatenate([rhs, lhs], axis=1).astype(np.float32)),
            'x0c': x0c,
        })

    shared = {
        'ce': cE,
        'ca': cA,
        'wb': wB,
        'bfc': bf16(bfcnp),
        'gall': bf16(gall),                            # [12, 512] bf16
    }

    tsim = bool(int(os.environ.get("TSIM", "0")))
    nc = bacc.Bacc("TRN2", target_bir_lowering=False, debug=False,
                   enable_asserts=False, num_devices=1 if tsim else NCORES)

    din = {}
    for k, v in shared.items():
        dt = bf16d if v.dtype == ml_dtypes.bfloat16 else f32
        din[k] = nc.dram_tensor(k, list(v.shape), dt, kind="ExternalInput")
    for k, v in per_core[0].items():
        dt = f32r if k == 'c3' else f32
        din[k] = nc.dram_tensor(k, list(v.shape), dt, kind="ExternalInput")
    out_d = nc.dram_tensor("out", [NMOL, 1], f32, kind="ExternalOutput")

    ar_in = [nc.dram_tensor(f"ar_in{b}", [N, H], f32, kind="Internal")
             for b in range(NB)]
    ar_out = [nc.dram_tensor(f"ar_out{b}", [SH, H], f32, kind="Internal")
              for b in range(NB)]
    hd_in = nc.dram_tensor("hd_in", [SH, NMOL], f32, kind="Internal")
    hd_out = nc.dram_tensor("hd_out", [SH, NMOL], f32, kind="Internal")
    RG = [list(range(NCORES))]

    AF = mybir.ActivationFunctionType
    AL = mybir.AluOpType

    with tile.TileContext(nc) as tc:
        with tc.tile_pool(name="const", bufs=1) as cpool, \
             tc.tile_pool(name="slab", bufs=1) as slabpool, \
             tc.tile_pool(name="y", bufs=1) as ypool, \
             tc.tile_pool(name="work", bufs=2) as wpool, \
             tc.tile_pool(name="h1s", bufs=1) as hpool, \
             tc.tile_pool(name="pro", bufs=2, space="PSUM") as propool, \
             tc.tile_pool(name="msc", bufs=1, space="PSUM") as miscpool, \
             tc.tile_pool(name="stp", bufs=1, space="PSUM") as stpool, \
             tc.tile_pool(name="aps", bufs=2, space="PSUM") as apool, \
             tc.tile_pool(name="acc", bufs=2, space="PSUM") as accpool:

            dummy = wpool.tile([1, 2], f32, tag="dummy")
            nc.gpsimd.memset(dummy[:], 1.0)

            # ---- input DMAs split across the SP and ACT HWDGE queues;
            # gall/ce first, big ca last so it can't starve block 0 ----
            c3 = cpool.tile([69, 576], f32r, tag="c3")
            nc.sync.dma_start(c3[:], din['c3'].ap())
            tgall = cpool.tile([64 + M, 4 * 128], bf16d, tag="tgall")
            nc.scalar.dma_start(tgall[64:64+M, :], din['gall'].ap())
            x0c = cpool.tile([128, SH + NMOL], f32, tag="x0c")
            nc.sync.dma_start(x0c[:], din['x0c'].ap())
            ce = cpool.tile(list(cE.shape), f32, tag="ce")
            nc.scalar.dma_start(ce[:], din['ce'].ap())
            bfc = cpool.tile(list(bfcnp.shape), bf16d, tag="bfc")
            nc.sync.dma_start(bfc[:], din['bfc'].ap())
            ca = cpool.tile(list(cA.shape), f32, tag="ca")
            nc.sync.dma_start(ca[:], din['ca'].ap())
            wb = cpool.tile(list(wB.shape), bf16d, tag="wb")
            nc.sync.dma_start(wb[:], din['wb'].ap())

            # views
            x0t = x0c[:, 0:SH]
            poolc = x0c[0:SH, SH:SH+NMOL]
            negkv = ce[:, 0:NGM]
            cev = {}
            off = 2 * NGM
            for name in ('w1x0', 'u1a0'):
                cev[name] = ce[:, off:off+128]
                off += 128
            h1bv = ce[:, off:off+4]; off += 4
            ob1v = ce[0:64, off:off+1]; off += 1
            o2v = ce[0:65, off:off+1]; off += 1
            for name in ('w1x1', 'u1a1'):
                cev[name] = ce[:, off:off+128]
                off += 128
            cav = {}
            off = 0
            for name in ('wu0', 'w1x2', 'w1x3', 'u1a2', 'u1a3',
                         'wu1', 'wu2', 'wu3'):
                cav[name] = ca[:, off:off+128]
                off += 128
            o1v = ca[:, off:off+64]; off += 64
            i128v = ca[:, off:off+128]; off += 128
            W1X = [cev['w1x0'], cev['w1x1'], cav['w1x2'], cav['w1x3']]
            U1A = [cev['u1a0'], cev['u1a1'], cav['u1a2'], cav['u1a3']]
            WU = [cav['wu0'], cav['wu1'], cav['wu2'], cav['wu3']]
            W2W1 = {p: wb[:, 128*i:128*(i+1)]
                    for i, p in enumerate(pairs)}
            UUv = {p: wb[:, 768+128*i:768+128*(i+1)]
                   for i, p in enumerate(pairs)}
            uov = [wb[:, 1536+64*b:1536+64*(b+1)] for b in range(NB)]
            bselv = bfc[0:SH, 0:256]
            eselv = bfc[:, 256:256+NT*128]

            # ---- distances: one matmul gives d^2 + BIG*I ----
            warm_ps = accpool.tile([SH, SH], f32, tag="acc")
            nc.tensor.matmul(warm_ps[:], c3[:, 512:576], c3[:, 512:576],
                             start=True, stop=True)
            d2_ps = accpool.tile([SH, N], f32, tag="acc")
            nc.tensor.matmul(d2_ps[:], c3[:, 512:576], c3[:, 0:512],
                             start=True, stop=True)
            # dm = sqrt(d^2)/delta = sqrt(d^2 / delta^2), straight from PSUM
            dm = wpool.tile([SH, N], bf16d, tag="dm")
            nc.scalar.activation(dm[:], d2_ps[:], AF.Sqrt,
                                 scale=1.0 / (delta * delta))

            # ---- hat slabs [128=(4m x 32i), 512 j] bf16, shared by blocks;
            # the ACT |.| stream interleaves with block Y silus via deps
            # gi=1 first: its bc bank is released early (its tiles run on
            # DVE/Pool) and the a_ps pair-2 ring slot lands on it
            bc_ps1 = propool.tile([128, N], f32, tag="bc")
            nc.tensor.matmul(bc_ps1[:], bselv[:, 128:256], dm[:],
                             start=True, stop=True)
            bc_ps0 = propool.tile([128, N], f32, tag="bc")
            nc.tensor.matmul(bc_ps0[:], bselv[:, 0:128], dm[:],
                             start=True, stop=True)
            bcs = [bc_ps0, bc_ps1]

            # stage1 |d~-k~| on ACT (Abs only lowers there); stage2
            # (min(x-1,0), bf16 2x) on DVE.  Abs lives in the SAME table
            # set as Sqrt (set "sqrt_and_others"), so running all six
            # before the first Silu keeps the 1283ns set switch out of
            # the slab stream; the dummy silu pinned to the LAST ug
            # triggers that switch right after Abs5.
            slabs = []
            for t in range(NT):
                gi, gm = t // NGM, t % NGM
                ug = wpool.tile([128, N], bf16d, tag=f"ug{t % 3}")
                nc.scalar.activation(ug[:], bcs[gi][:], AF.Abs,
                                     bias=negkv[:, gm:gm+1])
                sl = slabpool.tile([128, N], bf16d, tag=f"slab{t}")
                nc.vector.tensor_scalar(sl[:], ug[:], 1.0, 0.0,
                                        AL.subtract, AL.min)
                slabs.append(sl)
                if t == NT - 1:
                    nc.scalar.activation(dummy[:, 0:1], ug[0:1, 0:1],
                                         AF.Silu)

            # ---- one shared PSUM bank for the short-lived accumulators.
            # Each region's accumulation window is DISJOINT in time (t at
            # block start, h1pre at S-time, head after block 3), so the
            # bank never holds two open accumulations at once.
            misc = miscpool.tile([128, 512], f32, tag="misc")
            t_ps = misc[0:SH, 0:128]
            h1_ps = misc[:, 128:192]
            xo_ps = misc[0:SH, 256:320]
            pool_ps = misc[0:SH, 320:336]
            o_ps = misc[0:1, 336:352]
            sTps = stpool.tile([128, 2 * SH], f32, tag="sTps")
            # cols 64:128 of the sT bank host the head xo accumulator in
            # the tsim build (its window opens after sT_3 closes)
            xo_ps2 = sTps[0:SH, SH:2*SH]

            h1t = []

            def emit_t(b):
                # t_b = x0 @ w1x_b + sum_a h1_a @ (u2_a @ w1x_b); all but
                # the last term have no h1_{b-1} dependency and hide under
                # the preceding silu/accumulation phase
                nc.tensor.matmul(t_ps, x0t, W1X[b], start=True,
                                 stop=(b == 0))
                for a in range(len(h1t)):
                    nc.tensor.matmul(t_ps, h1t[a][:], W2W1[(a, b)],
                                     start=False, stop=(a == b - 1))

            emit_t(0)

            pending_stage = []
            pending_acc = []
            for b in range(NB):
                # tg rows 0:64 <- t_b (PSUM), rows 64:76 hold G'_b (DMA'd)
                nc.vector.tensor_copy(tgall[0:64, 128*b:128*(b+1)], t_ps)

                # Y knot values, triple-packed: 2 x ([128,384] psum, one
                # silu) — shortest silu cascade that still fits one bank
                ys = []
                a_pss = []

                def emit_tri(tri):
                    a_ps = apool.tile([128, 384], f32, tag="a")
                    a_pss.append(a_ps)
                    for third in range(3):
                        g = 3 * tri + third
                        nc.tensor.matmul(a_ps[:, 128*third:128*(third+1)],
                                         eselv[:, 128*g:128*(g+1)],
                                         tgall[:, 128*b:128*(b+1)],
                                         start=True, stop=True)
                    yt = ypool.tile([128, 384], bf16d, tag=f"y{tri}")
                    nc.scalar.activation(yt[:], a_pss[tri][:], AF.Silu)
                    ys.append(yt)

                emit_tri(0)
                emit_tri(1)
                # previous block's deferred RS-chunk matmuls + staging run
                # in the silu bubble
                while pending_acc:
                    pending_acc.pop()()
                while pending_stage:
                    pending_stage.pop()()

                # aggregate into [j, h]: one packed PSUM bank, 4 j-chunks,
                # iterated CHUNK-major — a PSUM bank holds only ONE open
                # accumulation window at a time (an interleaved start
                # resets the bank's accumulation context; measured on hw).
                # The own-shard sT matmuls go first: they gate S and the
                # serial chain; the full-width chunks only feed the staged
                # ReduceScatter and drain under the next block's Y phase.
                acc = accpool.tile([128, 512], f32, tag="acc")
                if tsim:
                    for g in range(NT):
                        yv = ys[g // 3][:, 128*(g % 3):128*(g % 3 + 1)]
                        nc.tensor.matmul(sTps[:, 0:SH], yv, slabs[g][:, 0:SH],
                                         start=(g == 0), stop=(g == NT - 1))

                def emit_chunk(q, acc=acc, ys=ys):
                    for g in range(NT):
                        yv = ys[g // 3][:, 128*(g % 3):128*(g % 3 + 1)]
                        nc.tensor.matmul(acc[:, 128*q:128*(q+1)],
                                         slabs[g][:, 128*q:128*(q+1)],
                                         yv, start=(g == 0),
                                         stop=(g == NT - 1))

                if tsim:
                    # full-width chunks only feed the staged ReduceScatter:
                    # run them under the NEXT block's silu bubble (or the
                    # head's stall windows) so the in-order PE reaches
                    # wu@S without 24 matmuls queued in front of it
                    for q in range(4):
                        pending_acc.append(
                            lambda q=q: emit_chunk(q))
                else:
                    for q in range(4):
                        emit_chunk(q)

                def stage_acc(on_act=False, b=b, acc=acc):
                    # stage the aggregate for the ReduceScatter (one copy
                    # + one DMA; gpsimd can't read PSUM so DVE — or ACT
                    # for the last block, keeping DVE clear for o_sb —
                    # moves it)
                    accsb = hpool.tile([128, 512], f32,
                                       tag=f"accsb{b % 2}")
                    if on_act:
                        nc.scalar.activation(accsb[:], acc[:], AF.Copy)
                    else:
                        nc.vector.tensor_copy(accsb[:], acc[:])
                    nc.sync.dma_start(
                        ar_in[b].ap().rearrange("(q p) h -> p q h", q=4),
                        accsb[:])

                if not tsim:
                    # the RS needs the staged aggregate now; in the tsim
                    # build S comes from sTps and staging is deferred past
                    # the next block's critical DVE copies
                    stage_acc()
                else:
                    pending_stage.append(stage_acc)
                S = wpool.tile([H, SH], f32, tag=f"S{b % 2}")
                if tsim:
                    nc.vector.tensor_copy(S[:], sTps[:, 0:SH])
                    if b == NB - 1:
                        # head xo: all terms except uo_3 hide here, in the
                        # freed sT bank (its window closed at g5)
                        nc.tensor.matmul(xo_ps2, x0t, o1v,
                                         start=True, stop=False)
                        for a in range(NB - 1):
                            nc.tensor.matmul(xo_ps2, h1t[a][:], uov[a],
                                             start=False, stop=False)
                else:
                    nc.gpsimd.collective_compute(
                        "ReduceScatter", AL.add, replica_groups=RG,
                        ins=[ar_in[b].ap()], outs=[ar_out[b].ap()])
                    s_jh = wpool.tile([SH, H], f32, tag=f"sjh{b % 2}")
                    nc.sync.dma_start(s_jh[:], ar_out[b].ap())
                    nc.tensor.transpose(sTps[:, 0:SH], s_jh[:],
                                        i128v[0:SH, 0:SH])
                    nc.vector.tensor_copy(S[:], sTps[:, 0:SH])

                # update MLP, just-in-time: h1pre = u1a@X + wu@S + bias
                # with X expanded in the h1 history; the x0/history terms
                # hide under the S copy, wu@S (stop) closes the window
                nc.tensor.matmul(h1_ps, U1A[b], x0t, start=True, stop=False)
                for a in range(len(h1t)):
                    nc.tensor.matmul(h1_ps, UUv[(a, b)], h1t[a][:],
                                     start=False, stop=False)
                nc.tensor.matmul(h1_ps, WU[b], S[:], start=False, stop=True)
                h1 = hpool.tile([H, SH], bf16d, tag=f"h1_{b}")
                nc.scalar.activation(h1[:], h1_ps, AF.Silu,
                                     bias=h1bv[:, b:b+1])
                h1t.append(h1)

                if b + 1 < NB:
                    emit_t(b + 1)

            # ---- head: xo = X^4 @ out_w1 via the h1 history, pool, MLP ----
            if tsim:
                nc.tensor.matmul(xo_ps2, h1t[NB - 1][:], uov[NB - 1],
                                 start=False, stop=True)
            else:
                nc.tensor.matmul(xo_ps, x0t, o1v, start=True, stop=False)
                for a in range(NB):
                    nc.tensor.matmul(xo_ps, h1t[a][:], uov[a],
                                     start=False, stop=(a == NB - 1))
            xo = wpool.tile([SH, 64], f32, tag="xov")
            nc.vector.tensor_copy(xo[:], xo_ps2 if tsim else xo_ps)
            if pending_acc:
                pending_acc.pop(0)()        # one chunk under the xo copy
            nc.tensor.matmul(pool_ps, xo[:], poolc, start=True, stop=True)
            while pending_acc:
                pending_acc.pop(0)()        # rest under the hh ACT window
            hh = wpool.tile([65, NMOL], f32, tag="hh")
            nc.gpsimd.memset(hh[64:65, :], float(out_b2[0]))
            if tsim:
                nc.scalar.activation(hh[0:64, :], pool_ps, AF.Silu,
                                     bias=ob1v)
            else:
                hps = wpool.tile([64, NMOL], f32, tag="hp_sb")
                nc.vector.tensor_copy(hps[:], pool_ps)
                nc.sync.dma_start(hd_in.ap(), hps[:])
                nc.gpsimd.collective_compute(
                    "AllReduce", AL.add, replica_groups=RG,
                    ins=[hd_in.ap()], outs=[hd_out.ap()])
                hpr = wpool.tile([64, NMOL], f32, tag="hp_r")
                nc.sync.dma_start(hpr[:], hd_out.ap())
                nc.scalar.activation(hh[0:64, :], hpr[:], AF.Silu,
                                     bias=ob1v)
            nc.tensor.matmul(o_ps, o2v, hh[:], start=True, stop=True)
            o_sb = wpool.tile([1, NMOL], f32, tag="o_sb")
            nc.vector.tensor_copy(o_sb[:], o_ps)
            nc.sync.dma_start(out_d.ap().rearrange("m one -> one m"),
                              o_sb[:])
            while pending_stage:
                pending_stage.pop()(True)   # block-3 staging, copy on ACT

    in_maps = []
    for c in range(NCORES):
        m = dict(shared)
        m.update(per_core[c])
        in_maps.append({k: np.ascontiguousarray(v) for k, v in m.items()})

    nc.compile()
    return nc, in_maps


def kernel(**inputs):
    import concourse.bass_utils as bass_utils
    nc, in_maps = build_nc(inputs)
    res = bass_utils.run_bass_kernel_spmd(nc, in_maps,
                                          core_ids=list(range(NCORES)))
    global LAST_EXEC_NS
    LAST_EXEC_NS = res.exec_time_ns
    return res.results[0]["out"].astype(np.float32)
